# revision 19
# baseline (speedup 1.0000x reference)
"""Trainium2 Bass kernel for DepthwiseXCorr (SiamRPN-style head).

Pipeline per batch sample:
  k = BN+ReLU(conv1x1(kernel, w_k))      [256, 7, 7]
  s = BN+ReLU(conv1x1(search, w_s))      [256, 31, 31]
  feat = depthwise_xcorr(s, k)           [256, 25, 25]
  y = BN+ReLU(conv1x1(feat, w_h1))
  out = conv1x1(y, w_h2) + bias          [20, 25, 25]

Sharding: pure data-parallel, batch 128 -> 16 per core across 8 cores.

Implementation notes:
- depthwise xcorr runs entirely on the PE in fp8-e4m3 DoubleRow mode
  (0.5 cycles/output-column: 2 contraction slots per partition).  The 49
  taps form 24 pairs + 1 singleton; a pair's stationary weight is a
  [128, 2, 128] tile holding diag(fp8(k[:, t0])), diag(fp8(k[:, t1])).
  For fp8 accuracy the search features are split into two fp8 planes
  s = s_hi + s_lo (s_lo = fp8(s - fp8(s))) and every pair is issued
  twice (hi windows + lo windows) accumulating in PSUM; the singleton
  puts (hi, lo) in its two slots with duplicated diag.
- The search conv and head conv1 also run fp8-DoubleRow: their operands
  are split hi/lo (weights host-side with a x16 scale folded into the
  BN scale; xcorr features on-chip), computed as the three-term product
  x_hi*w_hi + x_lo*w_hi + x_hi*w_lo (~bf16 accuracy, 0.75 cycles/col
  vs 2 for the bf16 two-chunk version).
- Measured end-to-end rel err ~1.77e-2 (tol 2e-2; all-bf16 is 5.1e-3).
- Per-(batch, chunk) diag builds (50 slots) spread over non-PE engines:
  DVE batched broadcast tensor_tensor, ScalarE per-tap scalar muls,
  GPSIMD batched affine_select; GPSIMD also casts s->fp8 (hi plane);
  DVE computes the s_lo / feat_lo residuals.
- BN+ReLU epilogues on ScalarE in f32; kernel conv + head conv2 bf16.
"""

import sys

if "/opt/trn_rl_repo" not in sys.path:
    sys.path.insert(0, "/opt/trn_rl_repo")

import ml_dtypes
import numpy as np

B, CIN, HID, COUT = 128, 256, 256, 20
NCORES = 8
NB = B // NCORES          # batches per core
HS = 31                   # search spatial
HK = 7                    # kernel spatial
HO = HS - HK + 1          # 25, xcorr output spatial
EPS = 1e-5
GB = 4                    # batch group size for the search-branch pipeline
NCH = 2                   # channel chunks of 128
NTAP = HK * HK            # 49
NPAIR = (NTAP + 1) // 2   # 25 (last is the singleton)
# diag-build distribution over engines, in pairs (2 slots each):
DVE_CHUNKS = [5, 5, 4]    # 14 pairs on DVE in 3 batched ops
ACT_PAIRS = 4             # 4 pairs (8 slots) as ScalarE singles
WSCALE = 16.0             # host scale for fp8 conv weights
BF16 = ml_dtypes.bfloat16
E4M3 = ml_dtypes.float8_e4m3fn

_CACHE = {}


def _build_nc(repeat=1):
    import concourse.bacc as bacc
    import concourse.tile as tile
    from concourse import mybir
    from concourse.ap import AP

    f32 = mybir.dt.float32
    bf16 = mybir.dt.bfloat16
    fp8 = mybir.dt.float8e4
    DR = mybir.MatmulPerfMode.DoubleRow

    nc = bacc.Bacc()

    xk = nc.dram_tensor("xk", [NB, CIN, HK, HK], bf16, kind="ExternalInput")
    xs_hi = nc.dram_tensor("xs_hi", [NB, CIN, HS, HS], fp8,
                           kind="ExternalInput")
    xs_lo = nc.dram_tensor("xs_lo", [NB, CIN, HS, HS], fp8,
                           kind="ExternalInput")
    wkT = nc.dram_tensor("wkT", [CIN, HID], bf16, kind="ExternalInput")
    ws_hi_d = nc.dram_tensor("ws_hi", [CIN, HID], fp8, kind="ExternalInput")
    ws_lo_d = nc.dram_tensor("ws_lo", [CIN, HID], fp8, kind="ExternalInput")
    wh1_hi_d = nc.dram_tensor("wh1_hi", [CIN, HID], fp8, kind="ExternalInput")
    wh1_lo_d = nc.dram_tensor("wh1_lo", [CIN, HID], fp8, kind="ExternalInput")
    wh2T = nc.dram_tensor("wh2T", [HID, COUT], bf16, kind="ExternalInput")
    sck = nc.dram_tensor("sck", [HID, 1], f32, kind="ExternalInput")
    shk = nc.dram_tensor("shk", [HID, 1], f32, kind="ExternalInput")
    scs = nc.dram_tensor("scs", [HID, 1], f32, kind="ExternalInput")
    shs = nc.dram_tensor("shs", [HID, 1], f32, kind="ExternalInput")
    sch = nc.dram_tensor("sch", [HID, 1], f32, kind="ExternalInput")
    shh = nc.dram_tensor("shh", [HID, 1], f32, kind="ExternalInput")
    bh2 = nc.dram_tensor("bh2", [COUT, 1], f32, kind="ExternalInput")
    ident = nc.dram_tensor("ident", [128, 128], bf16, kind="ExternalInput")
    out = nc.dram_tensor("out", [NB, COUT, HO, HO], f32, kind="ExternalOutput")

    relu = mybir.ActivationFunctionType.Relu
    idfn = mybir.ActivationFunctionType.Identity
    copyfn = mybir.ActivationFunctionType.Copy

    def tap_uv(t):
        return t // HK, t % HK

    def pair_rhs(sf, bl, plane, p, r0, r1):
        """rhs AP [128, 2, r1-r0, HO] for tap pair p over one fp8 plane.

        For the singleton pair (p == NPAIR-1) the two slots are the
        (hi, lo) planes of the same tap; `plane` is ignored.
        """
        t0 = 2 * p
        u0, v0 = tap_uv(t0)
        if p == NPAIR - 1:
            plane, delta = 0, HS * HS
        else:
            u1, v1 = tap_uv(t0 + 1)
            delta = (u1 - u0) * HS + (v1 - v0)
        w = sf[:, bl, plane, u0 + r0:u0 + r1, v0:v0 + HO]
        return AP(tensor=w.tensor, offset=w.offset,
                  ap=[list(w.ap[0]), [delta, 2], [HS, r1 - r0], [1, HO]])

    with tile.TileContext(nc) as tc:
        with (
            tc.tile_pool(name="const", bufs=1) as cpool,
            tc.tile_pool(name="act", bufs=1) as apool,
            tc.tile_pool(name="stream", bufs=2) as spool,
            tc.tile_pool(name="diagp", bufs=3) as dpool,
            tc.tile_pool(name="psum", bufs=1, space="PSUM") as ppool,
        ):
            # ---- constants (critical-path order) ---------------------------
            wk_t = []
            sck_t, shk_t, scs_t, shs_t, sch_t, shh_t = [], [], [], [], [], []

            def _vec(vec_d, lst, nm, kc, sl):
                v = cpool.tile([128, 1], f32, name=f"{nm}_{kc}")
                nc.sync.dma_start(v[:], vec_d[sl, :])
                lst.append(v)

            for kc in range(NCH):
                sl = slice(kc * 128, (kc + 1) * 128)
                w1 = cpool.tile([128, HID], bf16, name=f"wk_{kc}")
                nc.sync.dma_start(w1[:], wkT[sl, :])
                wk_t.append(w1)
                _vec(sck, sck_t, "sck", kc, sl)
                _vec(shk, shk_t, "shk", kc, sl)
            id_t = cpool.tile([128, 128], bf16, name="id_t")
            nc.sync.dma_start(id_t[:], ident[:])
            xk_ap = xk[:].rearrange("b c h w -> c b (h w)")
            xk_t = []
            for kc in range(NCH):
                t = apool.tile([128, NB, HK * HK], bf16, name=f"xk_t{kc}")
                nc.gpsimd.dma_start(t[:], xk_ap[kc * 128:(kc + 1) * 128])
                xk_t.append(t)

            # search-conv weights (fp8 hi/lo) + BN vectors
            ws_t = {}
            for nm, dr in (("ws_hi", ws_hi_d), ("ws_lo", ws_lo_d)):
                w = cpool.tile([128, NCH, HID], fp8, name=nm + "_t")
                nc.sync.dma_start(w[:], dr[:].rearrange("(i p) m -> p i m",
                                                        i=NCH))
                ws_t[nm] = w
            for kc in range(NCH):
                sl = slice(kc * 128, (kc + 1) * 128)
                _vec(scs, scs_t, "scs", kc, sl)
                _vec(shs, shs_t, "shs", kc, sl)

            xs_hi_ap = xs_hi[:].rearrange("b (i p) h w -> p i b (h w)", i=NCH)
            xs_lo_ap = xs_lo[:].rearrange("b (i p) h w -> p i b (h w)", i=NCH)

            def load_xs(g):
                gsl = slice(g * GB, (g + 1) * GB)
                ts = []
                for nm, ap_ in (("hi", xs_hi_ap), ("lo", xs_lo_ap)):
                    t = spool.tile([128, NCH, GB, HS * HS], fp8,
                                   name=f"xs_{nm}", tag=f"xs{nm}")
                    for i in range(NCH):
                        nc.gpsimd.dma_start(t[:, i], ap_[:, i, gsl])
                    ts.append(t)
                return ts

            xs_g0 = load_xs(0)

            # ---- kernel branch conv (all NB batches at once) ---------------
            k_feat = []
            for mc in range(NCH):
                kf = apool.tile([128, NB, HK * HK], f32, name=f"k_feat{mc}")
                for half in range(2):
                    bs = slice(half * (NB // 2), (half + 1) * (NB // 2))
                    ps = ppool.tile([128, NB // 2, HK * HK], f32,
                                    name="ps_cs", tag="csA", bufs=1)
                    for kc in range(NCH):
                        nc.tensor.matmul(
                            ps[:],
                            wk_t[kc][:, mc * 128:(mc + 1) * 128],
                            xk_t[kc][:, bs, :],
                            start=(kc == 0), stop=(kc == NCH - 1),
                        )
                    nc.scalar.activation(kf[:, bs, :], ps[:], relu,
                                         bias=shk_t[mc][:], scale=sck_t[mc][:])
                k_feat.append(kf)

            # head weights (fp8 hi/lo for conv1, bf16 for conv2) + BN
            wh1_t, wh2_t = {}, []
            for nm, dr in (("wh1_hi", wh1_hi_d), ("wh1_lo", wh1_lo_d)):
                w = cpool.tile([128, NCH, HID], fp8, name=nm + "_t")
                nc.sync.dma_start(w[:], dr[:].rearrange("(i p) m -> p i m",
                                                        i=NCH))
                wh1_t[nm] = w
            for kc in range(NCH):
                sl = slice(kc * 128, (kc + 1) * 128)
                w4 = cpool.tile([128, COUT], bf16, name=f"wh2_{kc}")
                nc.sync.dma_start(w4[:], wh2T[sl, :])
                wh2_t.append(w4)
                _vec(sch, sch_t, "sch", kc, sl)
                _vec(shh, shh_t, "shh", kc, sl)
            bh2_t = cpool.tile([COUT, 1], f32, name="bh2_t")
            nc.sync.dma_start(bh2_t[:], bh2[:])

            for _rep in range(repeat):
              NGRP = NB // GB

              def emit_diag_pairs(b_abs, mc):
                  """Build the [128, NPAIR, 2, 128] fp8 diag-slot weights."""
                  kf = k_feat[mc]
                  dall = dpool.tile([128, NPAIR, 2, 128], fp8,
                                    name="dall", tag="diag", bufs=3)
                  p0 = 0
                  # DVE: batched broadcast tensor_tensor builds
                  for ck in DVE_CHUNKS:
                      o = dall[:, p0:p0 + ck].rearrange("p k t f -> p (k t) f")
                      i1 = id_t[:].unsqueeze(1).broadcast_to([128, 2 * ck, 128])
                      i2 = kf[:, b_abs, 2 * p0:2 * (p0 + ck)] \
                          .unsqueeze(-1).broadcast_to([128, 2 * ck, 128])
                      nc.vector.tensor_tensor(o, i1, i2,
                                              mybir.AluOpType.mult)
                      p0 += ck
                  # ScalarE: per-slot scalar muls (incl. the singleton's
                  # duplicated second slot)
                  for p in range(p0, p0 + ACT_PAIRS):
                      for j in range(2):
                          nc.scalar.mul(dall[:, p, j, :], id_t[:],
                                        kf[:, b_abs, 2 * p + j:2 * p + j + 1])
                  p0 += ACT_PAIRS
                  nc.scalar.mul(dall[:, NPAIR - 1, 1, :], id_t[:],
                                kf[:, b_abs, NTAP - 1:NTAP])
                  # GPSIMD: batched affine_select for taps 2*p0 .. 48
                  # (through the singleton's first slot)
                  ntap_pool = 2 * (NPAIR - p0) - 1
                  ob = dall[:, p0, 0, :]
                  o = AP(tensor=ob.tensor, offset=ob.offset,
                         ap=[list(ob.ap[0]), [128, ntap_pool], [1, 128]])
                  i2 = kf[:, b_abs, 2 * p0:2 * p0 + ntap_pool] \
                      .unsqueeze(-1).broadcast_to([128, ntap_pool, 128])
                  nc.gpsimd.affine_select(
                      o, i2, pattern=[[0, ntap_pool], [1, 128]],
                      compare_op=mybir.AluOpType.is_equal, fill=0.0,
                      base=0, channel_multiplier=-1)
                  return dall

              feat_lo_of = {}
              ys = {}

              def emit_group_conv(g, xs_t=None):
                  """search conv for one batch group (fp8 three-term DR).

                  Produces sf[mc]: [128, GB, 2, HS, HS] fp8 (hi/lo planes).
                  """
                  if xs_t is None:
                      xs_t = load_xs(g)
                  xt_hi, xt_lo = xs_t
                  sf_g = [spool.tile([128, GB, 2, HS, HS], fp8,
                                     name=f"sf{mc}", tag=f"sf{mc}")
                          for mc in range(NCH)]
                  for bl in range(GB):
                      for mc in range(NCH):
                          msl = slice(mc * 128, (mc + 1) * 128)
                          sf = sf_g[mc]
                          s32 = apool.tile([128, HS, HS], f32, name="s32",
                                           tag="s32", bufs=4)
                          for ph, (r0, r1) in enumerate(((0, 16), (16, 31))):
                              ps = ppool.tile([128, r1 - r0, HS], f32,
                                              name="ps_cs2",
                                              tag=("csA" if ph == 0 else "csB"),
                                              bufs=1)
                              csl = slice(r0 * HS, r1 * HS)
                              terms = ((ws_t["ws_hi"], xt_hi),
                                       (ws_t["ws_lo"], xt_hi),
                                       (ws_t["ws_hi"], xt_lo))
                              for j, (wt, xt) in enumerate(terms):
                                  nc.tensor.matmul(
                                      ps[:], wt[:, :, msl],
                                      xt[:, :, bl, csl],
                                      start=(j == 0), stop=(j == 2),
                                      perf_mode=DR,
                                  )
                              nc.scalar.activation(s32[:, r0:r1, :], ps[:],
                                                   relu, bias=shs_t[mc][:],
                                                   scale=scs_t[mc][:])
                          # hi plane: fp8 cast on GPSIMD
                          nc.gpsimd.tensor_copy(sf[:, bl, 0], s32[:])
                          # lo plane: residual on DVE
                          nc.vector.tensor_tensor(sf[:, bl, 1], s32[:],
                                                  sf[:, bl, 0],
                                                  mybir.AluOpType.subtract)
                  return sf_g

              sf_cur = emit_group_conv(0, xs_g0 if _rep == 0 else None)
              for g in range(NGRP):
                  sf_next = None
                  for bl in range(GB):
                      if bl == GB - 2 and g + 1 < NGRP:
                          sf_next = emit_group_conv(g + 1)
                      b_abs = g * GB + bl
                      ftall = apool.tile([128, NCH, 2, HO * HO], fp8,
                                         name="ftall", tag="f", bufs=3)
                      for mc in range(NCH):
                          dall = emit_diag_pairs(b_abs, mc)
                          for ph, (r0, r1) in enumerate(((0, 20), (20, 25))):
                              psx = ppool.tile([128, (r1 - r0) * HO], f32,
                                               name="ps_xc", tag="xc", bufs=3)
                              nmm, last = 0, 2 * (NPAIR - 1)
                              for p in range(NPAIR - 1):
                                  for plane in range(2):
                                      nc.tensor.matmul(
                                          psx[:], dall[:, p],
                                          pair_rhs(sf_cur[mc], bl, plane,
                                                   p, r0, r1),
                                          start=(nmm == 0), stop=False,
                                          perf_mode=DR,
                                      )
                                      nmm += 1
                              nc.tensor.matmul(
                                  psx[:], dall[:, NPAIR - 1],
                                  pair_rhs(sf_cur[mc], bl, 0, NPAIR - 1,
                                           r0, r1),
                                  start=False, stop=True, perf_mode=DR,
                              )
                              csl = slice(r0 * HO, r1 * HO)
                              nc.scalar.activation(ftall[:, mc, 0, csl],
                                                   psx[:], copyfn)
                              nc.vector.tensor_tensor(
                                  ftall[:, mc, 1, csl], psx[:],
                                  ftall[:, mc, 0, csl],
                                  mybir.AluOpType.subtract)

                          if mc < NCH - 1:
                              continue

                          # head conv1 (fp8 three-term DR) + BN/ReLU
                          for mq in range(NCH):
                              msl = slice(mq * 128, (mq + 1) * 128)
                              yt = apool.tile([128, HO * HO], bf16,
                                              name=f"y{mq}", tag=f"y{mq}",
                                              bufs=2)
                              for ph, (c0, c1) in enumerate(((0, 500),
                                                            (500, 625))):
                                  ps = ppool.tile([128, c1 - c0], f32,
                                                  name="ps_hd",
                                                  tag=("hdA" if ph == 0
                                                       else "hdB"),
                                                  bufs=(2 if ph == 0 else 1))
                                  terms = (
                                      (wh1_t["wh1_hi"], 0),
                                      (wh1_t["wh1_hi"], 1),
                                      (wh1_t["wh1_lo"], 0),
                                  )
                                  for j, (wt, plane) in enumerate(terms):
                                      nc.tensor.matmul(
                                          ps[:], wt[:, :, msl],
                                          ftall[:, :, plane, c0:c1],
                                          start=(j == 0), stop=(j == 2),
                                          perf_mode=DR,
                                      )
                                  nc.scalar.activation(yt[:, c0:c1], ps[:],
                                                       relu, bias=shh_t[mq][:],
                                                       scale=sch_t[mq][:])
                              ys[mq] = yt

                          # head conv2 + bias (bf16)
                          ot = apool.tile([COUT, HO * HO], f32,
                                          name="ot", tag="ot", bufs=2)
                          for ph, (c0, c1) in enumerate(((0, 500), (500, 625))):
                              ps = ppool.tile([COUT, c1 - c0], f32,
                                              name="ps_o",
                                              tag=("hdA" if ph == 0 else "hdB"),
                                              bufs=(2 if ph == 0 else 1))
                              for kc in range(NCH):
                                  nc.tensor.matmul(
                                      ps[:],
                                      wh2_t[kc][:],
                                      ys[kc][:, c0:c1],
                                      start=(kc == 0), stop=(kc == NCH - 1),
                                  )
                              nc.scalar.activation(ot[:, c0:c1], ps[:], idfn,
                                                   bias=bh2_t[:], scale=1.0)
                              nc.sync.dma_start(
                                  out[b_abs][:].rearrange(
                                      "o h w -> o (h w)")[:, c0:c1],
                                  ot[:, c0:c1])
                  if sf_next is not None:
                      sf_cur = sf_next

    nc.compile()
    return nc


def _get_nc():
    if "nc" not in _CACHE:
        _CACHE["nc"] = _build_nc()
    return _CACHE["nc"]


def _split8(a):
    hi = a.astype(E4M3)
    lo = (a - hi.astype(np.float32)).astype(E4M3)
    return hi, lo


def _prep_in_maps(inputs):
    """Host-side input prep shared by kernel() and the test harness."""
    def fold(g, b, m, v):
        sc = (np.asarray(g) / np.sqrt(np.asarray(v) + EPS)).astype(np.float32)
        sh = (np.asarray(b) - np.asarray(m) * sc).astype(np.float32)
        return sc.reshape(-1, 1), sh.reshape(-1, 1)

    sck, shk = fold(inputs["g_k"], inputs["b_k"], inputs["m_k"], inputs["v_k"])
    scs, shs = fold(inputs["g_s"], inputs["b_s"], inputs["m_s"], inputs["v_s"])
    sch, shh = fold(inputs["g_h"], inputs["b_h"], inputs["m_h"], inputs["v_h"])

    w_k = np.asarray(inputs["w_k"])
    w_s = np.asarray(inputs["w_s"])
    w_h1 = np.asarray(inputs["w_h1"])
    w_h2 = np.asarray(inputs["w_h2"])
    bias_h2 = np.asarray(inputs["bias_h2"])

    ws16 = (np.ascontiguousarray(w_s.T).astype(BF16).astype(np.float32)
            * WSCALE)
    ws_hi, ws_lo = _split8(ws16)
    wh116 = (np.ascontiguousarray(w_h1.T).astype(BF16).astype(np.float32)
             * WSCALE)
    wh1_hi, wh1_lo = _split8(wh116)

    common = {
        "wkT": np.ascontiguousarray(w_k.T).astype(BF16),
        "ws_hi": ws_hi, "ws_lo": ws_lo,
        "wh1_hi": wh1_hi, "wh1_lo": wh1_lo,
        "wh2T": np.ascontiguousarray(w_h2.T).astype(BF16),
        "sck": sck, "shk": shk,
        "scs": scs / WSCALE, "shs": shs,
        "sch": sch / WSCALE, "shh": shh,
        "bh2": bias_h2.astype(np.float32).reshape(-1, 1),
        "ident": np.eye(128, dtype=BF16),
    }
    xk_all = np.asarray(inputs["kernel"]).astype(BF16)
    xs16 = np.asarray(inputs["search"]).astype(BF16).astype(np.float32)
    xs_hi_all, xs_lo_all = _split8(xs16)

    in_maps = []
    for i in range(NCORES):
        bs = slice(i * NB, (i + 1) * NB)
        m = dict(common)
        m["xk"] = np.ascontiguousarray(xk_all[bs])
        m["xs_hi"] = np.ascontiguousarray(xs_hi_all[bs])
        m["xs_lo"] = np.ascontiguousarray(xs_lo_all[bs])
        in_maps.append(m)
    return in_maps


def kernel(kernel, search, w_k, g_k, b_k, m_k, v_k, w_s, g_s, b_s, m_s, v_s,
           w_h1, g_h, b_h, m_h, v_h, w_h2, bias_h2):
    from concourse.bass_utils import run_bass_kernel_spmd

    inputs = dict(kernel=kernel, search=search, w_k=w_k, g_k=g_k, b_k=b_k,
                  m_k=m_k, v_k=v_k, w_s=w_s, g_s=g_s, b_s=b_s, m_s=m_s,
                  v_s=v_s, w_h1=w_h1, g_h=g_h, b_h=b_h, m_h=m_h, v_h=v_h,
                  w_h2=w_h2, bias_h2=bias_h2)
    in_maps = _prep_in_maps(inputs)
    nc = _get_nc()
    res = run_bass_kernel_spmd(nc, in_maps, core_ids=list(range(NCORES)))
    return np.concatenate([res.results[i]["out"] for i in range(NCORES)],
                          axis=0)


# revision 31
# speedup vs baseline: 1.0026x; 1.0026x over previous
"""Trainium2 Bass kernel for DepthwiseXCorr (SiamRPN-style head).

Pipeline per batch sample:
  k = BN+ReLU(conv1x1(kernel, w_k))      [256, 7, 7]
  s = BN+ReLU(conv1x1(search, w_s))      [256, 31, 31]
  feat = depthwise_xcorr(s, k)           [256, 25, 25]
  y = BN+ReLU(conv1x1(feat, w_h1))
  out = conv1x1(y, w_h2) + bias          [20, 25, 25]

Sharding: pure data-parallel, batch 128 -> 16 per core across 8 cores.

Implementation notes:
- depthwise xcorr runs entirely on the PE in fp8-e4m3 DoubleRow mode
  (0.5 cycles/output-column: 2 contraction slots per partition).  The 49
  taps form 24 pairs + 1 singleton; a pair's stationary weight is a
  [128, 2, 128] tile holding diag(fp8(k[:, t0])), diag(fp8(k[:, t1])).
  For fp8 accuracy the search features are split into two fp8 planes
  s = s_hi + s_lo (s_lo = fp8(s - fp8(s))) and every pair is issued
  twice (hi windows + lo windows) accumulating in PSUM; the singleton
  puts (hi, lo) in its two slots with duplicated diag.
- The search conv and head conv1 also run fp8-DoubleRow: their operands
  are split hi/lo (weights host-side with a x16 scale folded into the
  BN scale; xcorr features on-chip), computed as the three-term product
  x_hi*w_hi + x_lo*w_hi + x_hi*w_lo (~bf16 accuracy, 0.75 cycles/col
  vs 2 for the bf16 two-chunk version).
- Measured end-to-end rel err ~1.69e-2 (tol 2e-2; all-bf16 is 5.1e-3).
- Per-(batch, chunk) diag builds (50 slots) spread over non-PE engines:
  DVE batched broadcast tensor_tensor, ScalarE per-tap scalar muls,
  GPSIMD batched affine_select; GPSIMD also casts s->fp8 (hi plane);
  DVE computes the s_lo / feat_lo residuals.
- BN+ReLU epilogues on ScalarE in f32; kernel conv + head conv2 bf16.
"""

import sys

if "/opt/trn_rl_repo" not in sys.path:
    sys.path.insert(0, "/opt/trn_rl_repo")

import ml_dtypes
import numpy as np

B, CIN, HID, COUT = 128, 256, 256, 20
NCORES = 8
NB = B // NCORES          # batches per core
HS = 31                   # search spatial
HK = 7                    # kernel spatial
HO = HS - HK + 1          # 25, xcorr output spatial
EPS = 1e-5
GB = 4                    # batch group size for the search-branch pipeline
NCH = 2                   # channel chunks of 128
NTAP = HK * HK            # 49
NPAIR = (NTAP + 1) // 2   # 25 (last is the singleton)
# diag-build distribution over engines, in pairs (2 slots each):
DVE_CHUNKS = [5, 5, 5]    # 14 pairs on DVE in 3 batched ops
ACT_PAIRS = 4             # 4 pairs (8 slots) as ScalarE singles
WSCALE = 16.0             # host scale for fp8 conv weights
BF16 = ml_dtypes.bfloat16
E4M3 = ml_dtypes.float8_e4m3fn

_CACHE = {}


def _build_nc(repeat=1):
    import concourse.bacc as bacc
    import concourse.tile as tile
    from concourse import mybir
    from concourse.ap import AP

    f32 = mybir.dt.float32
    bf16 = mybir.dt.bfloat16
    fp8 = mybir.dt.float8e4
    DR = mybir.MatmulPerfMode.DoubleRow

    nc = bacc.Bacc()

    xk = nc.dram_tensor("xk", [NB, CIN, HK, HK], bf16, kind="ExternalInput")
    xs_hi = nc.dram_tensor("xs_hi", [NB, CIN, HS, HS], fp8,
                           kind="ExternalInput")
    xs_lo = nc.dram_tensor("xs_lo", [NB, CIN, HS, HS], fp8,
                           kind="ExternalInput")
    wkT = nc.dram_tensor("wkT", [CIN, HID], bf16, kind="ExternalInput")
    ws_hi_d = nc.dram_tensor("ws_hi", [CIN, HID], fp8, kind="ExternalInput")
    ws_lo_d = nc.dram_tensor("ws_lo", [CIN, HID], fp8, kind="ExternalInput")
    wh1_hi_d = nc.dram_tensor("wh1_hi", [CIN, HID], fp8, kind="ExternalInput")
    wh1_lo_d = nc.dram_tensor("wh1_lo", [CIN, HID], fp8, kind="ExternalInput")
    wh2T = nc.dram_tensor("wh2T", [HID, COUT], bf16, kind="ExternalInput")
    sck = nc.dram_tensor("sck", [HID, 1], f32, kind="ExternalInput")
    shk = nc.dram_tensor("shk", [HID, 1], f32, kind="ExternalInput")
    scs = nc.dram_tensor("scs", [HID, 1], f32, kind="ExternalInput")
    shs = nc.dram_tensor("shs", [HID, 1], f32, kind="ExternalInput")
    sch = nc.dram_tensor("sch", [HID, 1], f32, kind="ExternalInput")
    shh = nc.dram_tensor("shh", [HID, 1], f32, kind="ExternalInput")
    bh2 = nc.dram_tensor("bh2", [COUT, 1], f32, kind="ExternalInput")
    ident = nc.dram_tensor("ident", [128, 128], bf16, kind="ExternalInput")
    out = nc.dram_tensor("out", [NB, COUT, HO, HO], f32, kind="ExternalOutput")

    relu = mybir.ActivationFunctionType.Relu
    idfn = mybir.ActivationFunctionType.Identity
    copyfn = mybir.ActivationFunctionType.Copy

    def tap_uv(t):
        return t // HK, t % HK

    def pair_rhs(sf, bl, plane, p, r0, r1):
        """rhs AP [128, 2, r1-r0, HO] for tap pair p over one fp8 plane.

        For the singleton pair (p == NPAIR-1) the two slots are the
        (hi, lo) planes of the same tap; `plane` is ignored.
        """
        t0 = 2 * p
        u0, v0 = tap_uv(t0)
        if p == NPAIR - 1:
            plane, delta = 0, HS * HS
        else:
            u1, v1 = tap_uv(t0 + 1)
            delta = (u1 - u0) * HS + (v1 - v0)
        w = sf[:, bl, plane, u0 + r0:u0 + r1, v0:v0 + HO]
        return AP(tensor=w.tensor, offset=w.offset,
                  ap=[list(w.ap[0]), [delta, 2], [HS, r1 - r0], [1, HO]])

    with tile.TileContext(nc) as tc:
        with (
            tc.tile_pool(name="const", bufs=1) as cpool,
            tc.tile_pool(name="act", bufs=1) as apool,
            tc.tile_pool(name="stream", bufs=2) as spool,
            tc.tile_pool(name="diagp", bufs=3) as dpool,
            tc.tile_pool(name="psum", bufs=1, space="PSUM") as ppool,
        ):
            # ---- constants (critical-path order) ---------------------------
            wk_t = []
            sck_t, shk_t, scs_t, shs_t, sch_t, shh_t = [], [], [], [], [], []

            def _vec(vec_d, lst, nm, kc, sl):
                v = cpool.tile([128, 1], f32, name=f"{nm}_{kc}")
                nc.sync.dma_start(v[:], vec_d[sl, :])
                lst.append(v)

            for kc in range(NCH):
                sl = slice(kc * 128, (kc + 1) * 128)
                w1 = cpool.tile([128, HID], bf16, name=f"wk_{kc}")
                nc.sync.dma_start(w1[:], wkT[sl, :])
                wk_t.append(w1)
                _vec(sck, sck_t, "sck", kc, sl)
                _vec(shk, shk_t, "shk", kc, sl)
            id_t = cpool.tile([128, 128], bf16, name="id_t")
            nc.sync.dma_start(id_t[:], ident[:])
            xk_ap = xk[:].rearrange("b c h w -> c b (h w)")
            xk_t = []
            for kc in range(NCH):
                t = apool.tile([128, NB, HK * HK], bf16, name=f"xk_t{kc}")
                nc.gpsimd.dma_start(t[:], xk_ap[kc * 128:(kc + 1) * 128])
                xk_t.append(t)

            # search-conv weights (fp8 hi/lo) + BN vectors
            ws_t = {}
            for nm, dr in (("ws_hi", ws_hi_d), ("ws_lo", ws_lo_d)):
                w = cpool.tile([128, NCH, HID], fp8, name=nm + "_t")
                nc.sync.dma_start(w[:], dr[:].rearrange("(i p) m -> p i m",
                                                        i=NCH))
                ws_t[nm] = w
            for kc in range(NCH):
                sl = slice(kc * 128, (kc + 1) * 128)
                _vec(scs, scs_t, "scs", kc, sl)
                _vec(shs, shs_t, "shs", kc, sl)

            xs_hi_ap = xs_hi[:].rearrange("b (i p) h w -> p i b (h w)", i=NCH)
            xs_lo_ap = xs_lo[:].rearrange("b (i p) h w -> p i b (h w)", i=NCH)

            def load_xs(g):
                gsl = slice(g * GB, (g + 1) * GB)
                ts = []
                for nm, ap_ in (("hi", xs_hi_ap), ("lo", xs_lo_ap)):
                    t = spool.tile([128, NCH, GB, HS * HS], fp8,
                                   name=f"xs_{nm}", tag=f"xs{nm}")
                    for i in range(NCH):
                        nc.gpsimd.dma_start(t[:, i], ap_[:, i, gsl])
                    ts.append(t)
                return ts

            xs_g0 = load_xs(0)

            # ---- kernel branch conv (all NB batches at once) ---------------
            k_feat = [apool.tile([128, NB, HK * HK], f32, name=f"k_feat{mc}")
                      for mc in range(NCH)]
            for mc in range(NCH):
                for half in range(2):
                    bs = slice(half * (NB // 2), (half + 1) * (NB // 2))
                    ps = ppool.tile([128, NB // 2, HK * HK], f32,
                                    name="ps_cs", tag="csA", bufs=1)
                    for kc in range(NCH):
                        nc.tensor.matmul(
                            ps[:],
                            wk_t[kc][:, mc * 128:(mc + 1) * 128],
                            xk_t[kc][:, bs, :],
                            start=(kc == 0), stop=(kc == NCH - 1),
                        )
                    nc.scalar.activation(k_feat[mc][:, bs, :], ps[:], relu,
                                         bias=shk_t[mc][:], scale=sck_t[mc][:])

            # head weights (fp8 hi/lo for conv1, bf16 for conv2) + BN
            wh1_t, wh2_t = {}, []
            for nm, dr in (("wh1_hi", wh1_hi_d), ("wh1_lo", wh1_lo_d)):
                w = cpool.tile([128, NCH, HID], fp8, name=nm + "_t")
                nc.sync.dma_start(w[:], dr[:].rearrange("(i p) m -> p i m",
                                                        i=NCH))
                wh1_t[nm] = w
            for kc in range(NCH):
                sl = slice(kc * 128, (kc + 1) * 128)
                w4 = cpool.tile([128, COUT], bf16, name=f"wh2_{kc}")
                nc.sync.dma_start(w4[:], wh2T[sl, :])
                wh2_t.append(w4)
                _vec(sch, sch_t, "sch", kc, sl)
                _vec(shh, shh_t, "shh", kc, sl)
            bh2_t = cpool.tile([COUT, 1], f32, name="bh2_t")
            nc.sync.dma_start(bh2_t[:], bh2[:])

            for _rep in range(repeat):
              NGRP = NB // GB

              def emit_diag_pairs(b_abs, mc):
                  """Build the [128, NPAIR, 2, 128] fp8 diag-slot weights."""
                  kf = k_feat[mc]
                  dall = dpool.tile([128, NPAIR, 2, 128], fp8,
                                    name="dall", tag="diag", bufs=3)
                  p0 = 0
                  # DVE: batched broadcast tensor_tensor builds
                  for ck in DVE_CHUNKS:
                      o = dall[:, p0:p0 + ck].rearrange("p k t f -> p (k t) f")
                      i1 = id_t[:].unsqueeze(1).broadcast_to([128, 2 * ck, 128])
                      i2 = kf[:, b_abs, 2 * p0:2 * (p0 + ck)] \
                          .unsqueeze(-1).broadcast_to([128, 2 * ck, 128])
                      nc.vector.tensor_tensor(o, i1, i2,
                                              mybir.AluOpType.mult)
                      p0 += ck
                  # ScalarE: per-slot scalar muls (incl. the singleton's
                  # duplicated second slot)
                  for p in range(p0, p0 + ACT_PAIRS):
                      for j in range(2):
                          nc.scalar.mul(dall[:, p, j, :], id_t[:],
                                        kf[:, b_abs, 2 * p + j:2 * p + j + 1])
                  p0 += ACT_PAIRS
                  nc.scalar.mul(dall[:, NPAIR - 1, 1, :], id_t[:],
                                kf[:, b_abs, NTAP - 1:NTAP])
                  # GPSIMD: batched affine_select for taps 2*p0 .. 48
                  # (through the singleton's first slot)
                  ntap_pool = 2 * (NPAIR - p0) - 1
                  ob = dall[:, p0, 0, :]
                  o = AP(tensor=ob.tensor, offset=ob.offset,
                         ap=[list(ob.ap[0]), [128, ntap_pool], [1, 128]])
                  i2 = kf[:, b_abs, 2 * p0:2 * p0 + ntap_pool] \
                      .unsqueeze(-1).broadcast_to([128, ntap_pool, 128])
                  nc.gpsimd.affine_select(
                      o, i2, pattern=[[0, ntap_pool], [1, 128]],
                      compare_op=mybir.AluOpType.is_equal, fill=0.0,
                      base=0, channel_multiplier=-1)
                  return dall

              ys = {}

              def emit_group_conv(g, xs_t=None):
                  """search conv for one batch group (fp8 three-term DR).

                  Produces sf[mc]: [128, GB, 2, HS, HS] fp8 (hi/lo planes).
                  """
                  if xs_t is None:
                      xs_t = load_xs(g)
                  xt_hi, xt_lo = xs_t
                  sf_g = [spool.tile([128, GB, 2, HS, HS], fp8,
                                     name=f"sf{mc}", tag=f"sf{mc}")
                          for mc in range(NCH)]
                  for bl in range(GB):
                      for mc in range(NCH):
                          msl = slice(mc * 128, (mc + 1) * 128)
                          sf = sf_g[mc]
                          s32 = apool.tile([128, HS, HS], f32, name="s32",
                                           tag="s32", bufs=4)
                          for ph, (r0, r1) in enumerate(((0, 16), (16, 31))):
                              ps = ppool.tile([128, r1 - r0, HS], f32,
                                              name="ps_cs2",
                                              tag=("csA" if ph == 0 else "csB"),
                                              bufs=1)
                              csl = slice(r0 * HS, r1 * HS)
                              terms = ((ws_t["ws_hi"], xt_hi),
                                       (ws_t["ws_lo"], xt_hi),
                                       (ws_t["ws_hi"], xt_lo))
                              for j, (wt, xt) in enumerate(terms):
                                  nc.tensor.matmul(
                                      ps[:], wt[:, :, msl],
                                      xt[:, :, bl, csl],
                                      start=(j == 0), stop=(j == 2),
                                      perf_mode=DR,
                                  )
                              nc.scalar.activation(s32[:, r0:r1, :], ps[:],
                                                   relu, bias=shs_t[mc][:],
                                                   scale=scs_t[mc][:])
                          # hi plane: fp8 cast on GPSIMD
                          nc.gpsimd.tensor_copy(sf[:, bl, 0], s32[:])
                          # lo plane: residual on DVE
                          nc.vector.tensor_tensor(sf[:, bl, 1], s32[:],
                                                  sf[:, bl, 0],
                                                  mybir.AluOpType.subtract)
                  return sf_g

              sf_cur = emit_group_conv(0, xs_g0 if _rep == 0 else None)
              for g in range(NGRP):
                  sf_next = None
                  for bl in range(GB):
                      if bl == GB - 2 and g + 1 < NGRP:
                          sf_next = emit_group_conv(g + 1)
                      b_abs = g * GB + bl
                      ftall = apool.tile([128, NCH, 2, HO * HO], fp8,
                                         name="ftall", tag="f", bufs=3)
                      for mc in range(NCH):
                          dall = emit_diag_pairs(b_abs, mc)
                          for ph, (r0, r1) in enumerate(((0, 20), (20, 25))):
                              psx = ppool.tile([128, (r1 - r0) * HO], f32,
                                               name="ps_xc", tag="xc", bufs=3)
                              nmm, last = 0, 2 * (NPAIR - 1)
                              for p in range(NPAIR - 1):
                                  for plane in range(2):
                                      nc.tensor.matmul(
                                          psx[:], dall[:, p],
                                          pair_rhs(sf_cur[mc], bl, plane,
                                                   p, r0, r1),
                                          start=(nmm == 0), stop=False,
                                          perf_mode=DR,
                                      )
                                      nmm += 1
                              nc.tensor.matmul(
                                  psx[:], dall[:, NPAIR - 1],
                                  pair_rhs(sf_cur[mc], bl, 0, NPAIR - 1,
                                           r0, r1),
                                  start=False, stop=True, perf_mode=DR,
                              )
                              csl = slice(r0 * HO, r1 * HO)
                              nc.scalar.activation(ftall[:, mc, 0, csl],
                                                   psx[:], copyfn)
                              nc.vector.tensor_tensor(
                                  ftall[:, mc, 1, csl], psx[:],
                                  ftall[:, mc, 0, csl],
                                  mybir.AluOpType.subtract)

                          if mc < NCH - 1:
                              continue

                          # head conv1 (fp8 three-term DR) + BN/ReLU
                          for mq in range(NCH):
                              msl = slice(mq * 128, (mq + 1) * 128)
                              yt = apool.tile([128, HO * HO], bf16,
                                              name=f"y{mq}", tag=f"y{mq}",
                                              bufs=2)
                              for ph, (c0, c1) in enumerate(((0, 500),
                                                            (500, 625))):
                                  ps = ppool.tile([128, c1 - c0], f32,
                                                  name="ps_hd",
                                                  tag=("hdA" if ph == 0
                                                       else "hdB"),
                                                  bufs=(2 if ph == 0 else 1))
                                  terms = (
                                      (wh1_t["wh1_hi"], 0),
                                      (wh1_t["wh1_hi"], 1),
                                      (wh1_t["wh1_lo"], 0),
                                  )
                                  for j, (wt, plane) in enumerate(terms):
                                      nc.tensor.matmul(
                                          ps[:], wt[:, :, msl],
                                          ftall[:, :, plane, c0:c1],
                                          start=(j == 0), stop=(j == 2),
                                          perf_mode=DR,
                                      )
                                  nc.scalar.activation(yt[:, c0:c1], ps[:],
                                                       relu, bias=shh_t[mq][:],
                                                       scale=sch_t[mq][:])
                              ys[mq] = yt

                          # head conv2 + bias (bf16)
                          ot = apool.tile([COUT, HO * HO], f32,
                                          name="ot", tag="ot", bufs=2)
                          for ph, (c0, c1) in enumerate(((0, 500), (500, 625))):
                              ps = ppool.tile([COUT, c1 - c0], f32,
                                              name="ps_o",
                                              tag=("hdA" if ph == 0 else "hdB"),
                                              bufs=(2 if ph == 0 else 1))
                              for kc in range(NCH):
                                  nc.tensor.matmul(
                                      ps[:],
                                      wh2_t[kc][:],
                                      ys[kc][:, c0:c1],
                                      start=(kc == 0), stop=(kc == NCH - 1),
                                  )
                              nc.scalar.activation(ot[:, c0:c1], ps[:], idfn,
                                                   bias=bh2_t[:], scale=1.0)
                              nc.sync.dma_start(
                                  out[b_abs][:].rearrange(
                                      "o h w -> o (h w)")[:, c0:c1],
                                  ot[:, c0:c1])
                  if sf_next is not None:
                      sf_cur = sf_next

    nc.compile()
    return nc


def _get_nc():
    if "nc" not in _CACHE:
        _CACHE["nc"] = _build_nc()
    return _CACHE["nc"]


def _split8(a):
    hi = a.astype(E4M3)
    lo = (a - hi.astype(np.float32)).astype(E4M3)
    return hi, lo


def _prep_in_maps(inputs):
    """Host-side input prep shared by kernel() and the test harness."""
    def fold(g, b, m, v):
        sc = (np.asarray(g) / np.sqrt(np.asarray(v) + EPS)).astype(np.float32)
        sh = (np.asarray(b) - np.asarray(m) * sc).astype(np.float32)
        return sc.reshape(-1, 1), sh.reshape(-1, 1)

    sck, shk = fold(inputs["g_k"], inputs["b_k"], inputs["m_k"], inputs["v_k"])
    scs, shs = fold(inputs["g_s"], inputs["b_s"], inputs["m_s"], inputs["v_s"])
    sch, shh = fold(inputs["g_h"], inputs["b_h"], inputs["m_h"], inputs["v_h"])

    w_k = np.asarray(inputs["w_k"])
    w_s = np.asarray(inputs["w_s"])
    w_h1 = np.asarray(inputs["w_h1"])
    w_h2 = np.asarray(inputs["w_h2"])
    bias_h2 = np.asarray(inputs["bias_h2"])

    ws16 = (np.ascontiguousarray(w_s.T).astype(BF16).astype(np.float32)
            * WSCALE)
    ws_hi, ws_lo = _split8(ws16)
    wh116 = (np.ascontiguousarray(w_h1.T).astype(BF16).astype(np.float32)
             * WSCALE)
    wh1_hi, wh1_lo = _split8(wh116)

    common = {
        "wkT": np.ascontiguousarray(w_k.T).astype(BF16),
        "ws_hi": ws_hi, "ws_lo": ws_lo,
        "wh1_hi": wh1_hi, "wh1_lo": wh1_lo,
        "wh2T": np.ascontiguousarray(w_h2.T).astype(BF16),
        "sck": sck, "shk": shk,
        "scs": scs / WSCALE, "shs": shs,
        "sch": sch / WSCALE, "shh": shh,
        "bh2": bias_h2.astype(np.float32).reshape(-1, 1),
        "ident": np.eye(128, dtype=BF16),
    }
    xk_all = np.asarray(inputs["kernel"]).astype(BF16)
    xs16 = np.asarray(inputs["search"]).astype(BF16).astype(np.float32)
    xs_hi_all, xs_lo_all = _split8(xs16)

    in_maps = []
    for i in range(NCORES):
        bs = slice(i * NB, (i + 1) * NB)
        m = dict(common)
        m["xk"] = np.ascontiguousarray(xk_all[bs])
        m["xs_hi"] = np.ascontiguousarray(xs_hi_all[bs])
        m["xs_lo"] = np.ascontiguousarray(xs_lo_all[bs])
        in_maps.append(m)
    return in_maps


def kernel(kernel, search, w_k, g_k, b_k, m_k, v_k, w_s, g_s, b_s, m_s, v_s,
           w_h1, g_h, b_h, m_h, v_h, w_h2, bias_h2):
    from concourse.bass_utils import run_bass_kernel_spmd

    inputs = dict(kernel=kernel, search=search, w_k=w_k, g_k=g_k, b_k=b_k,
                  m_k=m_k, v_k=v_k, w_s=w_s, g_s=g_s, b_s=b_s, m_s=m_s,
                  v_s=v_s, w_h1=w_h1, g_h=g_h, b_h=b_h, m_h=m_h, v_h=v_h,
                  w_h2=w_h2, bias_h2=bias_h2)
    in_maps = _prep_in_maps(inputs)
    nc = _get_nc()
    last_err = None
    for _attempt in range(2):
        try:
            res = run_bass_kernel_spmd(nc, in_maps,
                                       core_ids=list(range(NCORES)))
            return np.concatenate(
                [res.results[i]["out"] for i in range(NCORES)], axis=0)
        except Exception as e:  # transient axon/mesh errors: retry
            last_err = e
    # Fallback: the batch shards are independent (no collectives), so a
    # wedged multi-core mesh can be routed around with per-core dispatches.
    try:
        outs = []
        for i in range(NCORES):
            res = run_bass_kernel_spmd(nc, [in_maps[i]], core_ids=[i])
            outs.append(res.results[0]["out"])
        return np.concatenate(outs, axis=0)
    except Exception:
        raise last_err


# revision 34
# speedup vs baseline: 1.0041x; 1.0015x over previous
"""Trainium2 Bass kernel for DepthwiseXCorr (SiamRPN-style head).

Pipeline per batch sample:
  k = BN+ReLU(conv1x1(kernel, w_k))      [256, 7, 7]
  s = BN+ReLU(conv1x1(search, w_s))      [256, 31, 31]
  feat = depthwise_xcorr(s, k)           [256, 25, 25]
  y = BN+ReLU(conv1x1(feat, w_h1))
  out = conv1x1(y, w_h2) + bias          [20, 25, 25]

Sharding: pure data-parallel, batch 128 -> 16 per core across 8 cores.

Implementation notes:
- depthwise xcorr runs entirely on the PE in fp8-e4m3 DoubleRow mode
  (0.5 cycles/output-column: 2 contraction slots per partition).  The 49
  taps form 24 pairs + 1 singleton; a pair's stationary weight is a
  [128, 2, 128] tile holding diag(fp8(k[:, t0])), diag(fp8(k[:, t1])).
  For fp8 accuracy the search features are split into two fp8 planes
  s = s_hi + s_lo (s_lo = fp8(s - fp8(s))) and every pair is issued
  twice (hi windows + lo windows) accumulating in PSUM; the singleton
  puts (hi, lo) in its two slots with duplicated diag.
- The search conv and head conv1 also run fp8-DoubleRow: their operands
  are split hi/lo (weights host-side with a x16 scale folded into the
  BN scale; xcorr features on-chip), computed as the three-term product
  x_hi*w_hi + x_lo*w_hi + x_hi*w_lo (~bf16 accuracy, 0.75 cycles/col
  vs 2 for the bf16 two-chunk version).
- Measured end-to-end rel err ~1.69e-2 (tol 2e-2; all-bf16 is 5.1e-3).
- Per-(batch, chunk) diag builds (50 slots) spread over non-PE engines:
  DVE batched broadcast tensor_tensor, ScalarE per-tap scalar muls,
  GPSIMD batched affine_select; GPSIMD also casts s->fp8 (hi plane);
  DVE computes the s_lo / feat_lo residuals.
- BN+ReLU epilogues on ScalarE in f32; kernel conv + head conv2 bf16.
"""

import sys

if "/opt/trn_rl_repo" not in sys.path:
    sys.path.insert(0, "/opt/trn_rl_repo")

import ml_dtypes
import numpy as np

B, CIN, HID, COUT = 128, 256, 256, 20
NCORES = 8
NB = B // NCORES          # batches per core
HS = 31                   # search spatial
HK = 7                    # kernel spatial
HO = HS - HK + 1          # 25, xcorr output spatial
EPS = 1e-5
GB = 2                    # batch group size for the search-branch pipeline
NCH = 2                   # channel chunks of 128
NTAP = HK * HK            # 49
NPAIR = (NTAP + 1) // 2   # 25 (last is the singleton)
# diag-build distribution over engines, in pairs (2 slots each):
DVE_CHUNKS = [5, 5, 5]    # 14 pairs on DVE in 3 batched ops
ACT_PAIRS = 4             # 4 pairs (8 slots) as ScalarE singles
WSCALE = 16.0             # host scale for fp8 conv weights
BF16 = ml_dtypes.bfloat16
E4M3 = ml_dtypes.float8_e4m3fn

_CACHE = {}


def _build_nc(repeat=1):
    import concourse.bacc as bacc
    import concourse.tile as tile
    from concourse import mybir
    from concourse.ap import AP

    f32 = mybir.dt.float32
    bf16 = mybir.dt.bfloat16
    fp8 = mybir.dt.float8e4
    DR = mybir.MatmulPerfMode.DoubleRow

    nc = bacc.Bacc()

    xk = nc.dram_tensor("xk", [NB, CIN, HK, HK], bf16, kind="ExternalInput")
    xs_hi = nc.dram_tensor("xs_hi", [NB, CIN, HS, HS], fp8,
                           kind="ExternalInput")
    xs_lo = nc.dram_tensor("xs_lo", [NB, CIN, HS, HS], fp8,
                           kind="ExternalInput")
    wkT = nc.dram_tensor("wkT", [CIN, HID], bf16, kind="ExternalInput")
    ws_hi_d = nc.dram_tensor("ws_hi", [CIN, HID], fp8, kind="ExternalInput")
    ws_lo_d = nc.dram_tensor("ws_lo", [CIN, HID], fp8, kind="ExternalInput")
    wh1_hi_d = nc.dram_tensor("wh1_hi", [CIN, HID], fp8, kind="ExternalInput")
    wh1_lo_d = nc.dram_tensor("wh1_lo", [CIN, HID], fp8, kind="ExternalInput")
    wh2T = nc.dram_tensor("wh2T", [HID, COUT], bf16, kind="ExternalInput")
    sck = nc.dram_tensor("sck", [HID, 1], f32, kind="ExternalInput")
    shk = nc.dram_tensor("shk", [HID, 1], f32, kind="ExternalInput")
    scs = nc.dram_tensor("scs", [HID, 1], f32, kind="ExternalInput")
    shs = nc.dram_tensor("shs", [HID, 1], f32, kind="ExternalInput")
    sch = nc.dram_tensor("sch", [HID, 1], f32, kind="ExternalInput")
    shh = nc.dram_tensor("shh", [HID, 1], f32, kind="ExternalInput")
    bh2 = nc.dram_tensor("bh2", [COUT, 1], f32, kind="ExternalInput")
    ident = nc.dram_tensor("ident", [128, 128], bf16, kind="ExternalInput")
    out = nc.dram_tensor("out", [NB, COUT, HO, HO], f32, kind="ExternalOutput")

    relu = mybir.ActivationFunctionType.Relu
    idfn = mybir.ActivationFunctionType.Identity
    copyfn = mybir.ActivationFunctionType.Copy

    def tap_uv(t):
        return t // HK, t % HK

    def pair_rhs(sf, bl, plane, p, r0, r1):
        """rhs AP [128, 2, r1-r0, HO] for tap pair p over one fp8 plane.

        For the singleton pair (p == NPAIR-1) the two slots are the
        (hi, lo) planes of the same tap; `plane` is ignored.
        """
        t0 = 2 * p
        u0, v0 = tap_uv(t0)
        if p == NPAIR - 1:
            plane, delta = 0, HS * HS
        else:
            u1, v1 = tap_uv(t0 + 1)
            delta = (u1 - u0) * HS + (v1 - v0)
        w = sf[:, bl, plane, u0 + r0:u0 + r1, v0:v0 + HO]
        return AP(tensor=w.tensor, offset=w.offset,
                  ap=[list(w.ap[0]), [delta, 2], [HS, r1 - r0], [1, HO]])

    with tile.TileContext(nc) as tc:
        with (
            tc.tile_pool(name="const", bufs=1) as cpool,
            tc.tile_pool(name="act", bufs=1) as apool,
            tc.tile_pool(name="stream", bufs=2) as spool,
            tc.tile_pool(name="diagp", bufs=3) as dpool,
            tc.tile_pool(name="psum", bufs=1, space="PSUM") as ppool,
        ):
            # ---- constants (critical-path order) ---------------------------
            wk_t = []
            sck_t, shk_t, scs_t, shs_t, sch_t, shh_t = [], [], [], [], [], []

            def _vec(vec_d, lst, nm, kc, sl):
                v = cpool.tile([128, 1], f32, name=f"{nm}_{kc}")
                nc.sync.dma_start(v[:], vec_d[sl, :])
                lst.append(v)

            for kc in range(NCH):
                sl = slice(kc * 128, (kc + 1) * 128)
                w1 = cpool.tile([128, HID], bf16, name=f"wk_{kc}")
                nc.sync.dma_start(w1[:], wkT[sl, :])
                wk_t.append(w1)
                _vec(sck, sck_t, "sck", kc, sl)
                _vec(shk, shk_t, "shk", kc, sl)
            id_t = cpool.tile([128, 128], bf16, name="id_t")
            nc.sync.dma_start(id_t[:], ident[:])
            xk_ap = xk[:].rearrange("b c h w -> c b (h w)")
            xk_t = []
            for kc in range(NCH):
                t = apool.tile([128, NB, HK * HK], bf16, name=f"xk_t{kc}")
                nc.gpsimd.dma_start(t[:], xk_ap[kc * 128:(kc + 1) * 128])
                xk_t.append(t)

            # search-conv weights (fp8 hi/lo) + BN vectors
            ws_t = {}
            for nm, dr in (("ws_hi", ws_hi_d), ("ws_lo", ws_lo_d)):
                w = cpool.tile([128, NCH, HID], fp8, name=nm + "_t")
                nc.sync.dma_start(w[:], dr[:].rearrange("(i p) m -> p i m",
                                                        i=NCH))
                ws_t[nm] = w
            for kc in range(NCH):
                sl = slice(kc * 128, (kc + 1) * 128)
                _vec(scs, scs_t, "scs", kc, sl)
                _vec(shs, shs_t, "shs", kc, sl)

            xs_hi_ap = xs_hi[:].rearrange("b (i p) h w -> p i b (h w)", i=NCH)
            xs_lo_ap = xs_lo[:].rearrange("b (i p) h w -> p i b (h w)", i=NCH)

            def load_xs(g):
                gsl = slice(g * GB, (g + 1) * GB)
                ts = []
                for nm, ap_ in (("hi", xs_hi_ap), ("lo", xs_lo_ap)):
                    t = spool.tile([128, NCH, GB, HS * HS], fp8,
                                   name=f"xs_{nm}", tag=f"xs{nm}")
                    for i in range(NCH):
                        nc.gpsimd.dma_start(t[:, i], ap_[:, i, gsl])
                    ts.append(t)
                return ts

            xs_g0 = load_xs(0)

            # ---- kernel branch conv (all NB batches at once) ---------------
            k_feat = [apool.tile([128, NB, HK * HK], f32, name=f"k_feat{mc}")
                      for mc in range(NCH)]
            for mc in range(NCH):
                for half in range(2):
                    bs = slice(half * (NB // 2), (half + 1) * (NB // 2))
                    ps = ppool.tile([128, NB // 2, HK * HK], f32,
                                    name="ps_cs", tag="csA", bufs=1)
                    for kc in range(NCH):
                        nc.tensor.matmul(
                            ps[:],
                            wk_t[kc][:, mc * 128:(mc + 1) * 128],
                            xk_t[kc][:, bs, :],
                            start=(kc == 0), stop=(kc == NCH - 1),
                        )
                    nc.scalar.activation(k_feat[mc][:, bs, :], ps[:], relu,
                                         bias=shk_t[mc][:], scale=sck_t[mc][:])

            # head weights (fp8 hi/lo for conv1, bf16 for conv2) + BN
            wh1_t, wh2_t = {}, []
            for nm, dr in (("wh1_hi", wh1_hi_d), ("wh1_lo", wh1_lo_d)):
                w = cpool.tile([128, NCH, HID], fp8, name=nm + "_t")
                nc.sync.dma_start(w[:], dr[:].rearrange("(i p) m -> p i m",
                                                        i=NCH))
                wh1_t[nm] = w
            for kc in range(NCH):
                sl = slice(kc * 128, (kc + 1) * 128)
                w4 = cpool.tile([128, COUT], bf16, name=f"wh2_{kc}")
                nc.sync.dma_start(w4[:], wh2T[sl, :])
                wh2_t.append(w4)
                _vec(sch, sch_t, "sch", kc, sl)
                _vec(shh, shh_t, "shh", kc, sl)
            bh2_t = cpool.tile([COUT, 1], f32, name="bh2_t")
            nc.sync.dma_start(bh2_t[:], bh2[:])

            for _rep in range(repeat):
              NGRP = NB // GB

              def emit_diag_pairs(b_abs, mc):
                  """Build the [128, NPAIR, 2, 128] fp8 diag-slot weights."""
                  kf = k_feat[mc]
                  dall = dpool.tile([128, NPAIR, 2, 128], fp8,
                                    name="dall", tag="diag", bufs=3)
                  p0 = 0
                  # DVE: batched broadcast tensor_tensor builds
                  for ck in DVE_CHUNKS:
                      o = dall[:, p0:p0 + ck].rearrange("p k t f -> p (k t) f")
                      i1 = id_t[:].unsqueeze(1).broadcast_to([128, 2 * ck, 128])
                      i2 = kf[:, b_abs, 2 * p0:2 * (p0 + ck)] \
                          .unsqueeze(-1).broadcast_to([128, 2 * ck, 128])
                      nc.vector.tensor_tensor(o, i1, i2,
                                              mybir.AluOpType.mult)
                      p0 += ck
                  # ScalarE: per-slot scalar muls (incl. the singleton's
                  # duplicated second slot)
                  for p in range(p0, p0 + ACT_PAIRS):
                      for j in range(2):
                          nc.scalar.mul(dall[:, p, j, :], id_t[:],
                                        kf[:, b_abs, 2 * p + j:2 * p + j + 1])
                  p0 += ACT_PAIRS
                  nc.scalar.mul(dall[:, NPAIR - 1, 1, :], id_t[:],
                                kf[:, b_abs, NTAP - 1:NTAP])
                  # GPSIMD: batched affine_select for taps 2*p0 .. 48
                  # (through the singleton's first slot)
                  ntap_pool = 2 * (NPAIR - p0) - 1
                  ob = dall[:, p0, 0, :]
                  o = AP(tensor=ob.tensor, offset=ob.offset,
                         ap=[list(ob.ap[0]), [128, ntap_pool], [1, 128]])
                  i2 = kf[:, b_abs, 2 * p0:2 * p0 + ntap_pool] \
                      .unsqueeze(-1).broadcast_to([128, ntap_pool, 128])
                  nc.gpsimd.affine_select(
                      o, i2, pattern=[[0, ntap_pool], [1, 128]],
                      compare_op=mybir.AluOpType.is_equal, fill=0.0,
                      base=0, channel_multiplier=-1)
                  return dall

              ys = {}

              def emit_group_conv(g, xs_t=None):
                  """search conv for one batch group (fp8 three-term DR).

                  Produces sf[mc]: [128, GB, 2, HS, HS] fp8 (hi/lo planes).
                  """
                  if xs_t is None:
                      xs_t = load_xs(g)
                  xt_hi, xt_lo = xs_t
                  sf_g = [spool.tile([128, GB, 2, HS, HS], fp8,
                                     name=f"sf{mc}", tag=f"sf{mc}")
                          for mc in range(NCH)]
                  for bl in range(GB):
                      for mc in range(NCH):
                          msl = slice(mc * 128, (mc + 1) * 128)
                          sf = sf_g[mc]
                          s32 = apool.tile([128, HS, HS], f32, name="s32",
                                           tag="s32", bufs=4)
                          for ph, (r0, r1) in enumerate(((0, 16), (16, 31))):
                              ps = ppool.tile([128, r1 - r0, HS], f32,
                                              name="ps_cs2",
                                              tag=("csA" if ph == 0 else "csB"),
                                              bufs=1)
                              csl = slice(r0 * HS, r1 * HS)
                              terms = ((ws_t["ws_hi"], xt_hi),
                                       (ws_t["ws_lo"], xt_hi),
                                       (ws_t["ws_hi"], xt_lo))
                              for j, (wt, xt) in enumerate(terms):
                                  nc.tensor.matmul(
                                      ps[:], wt[:, :, msl],
                                      xt[:, :, bl, csl],
                                      start=(j == 0), stop=(j == 2),
                                      perf_mode=DR,
                                  )
                              nc.scalar.activation(s32[:, r0:r1, :], ps[:],
                                                   relu, bias=shs_t[mc][:],
                                                   scale=scs_t[mc][:])
                          # hi plane: fp8 cast on GPSIMD
                          nc.gpsimd.tensor_copy(sf[:, bl, 0], s32[:])
                          # lo plane: residual on DVE
                          nc.vector.tensor_tensor(sf[:, bl, 1], s32[:],
                                                  sf[:, bl, 0],
                                                  mybir.AluOpType.subtract)
                  return sf_g

              sf_cur = emit_group_conv(0, xs_g0 if _rep == 0 else None)
              for g in range(NGRP):
                  sf_next = None
                  for bl in range(GB):
                      if bl == GB - 2 and g + 1 < NGRP:
                          sf_next = emit_group_conv(g + 1)
                      b_abs = g * GB + bl
                      ftall = apool.tile([128, NCH, 2, HO * HO], fp8,
                                         name="ftall", tag="f", bufs=3)
                      for mc in range(NCH):
                          dall = emit_diag_pairs(b_abs, mc)
                          for ph, (r0, r1) in enumerate(((0, 20), (20, 25))):
                              psx = ppool.tile([128, (r1 - r0) * HO], f32,
                                               name="ps_xc", tag="xc", bufs=3)
                              nmm, last = 0, 2 * (NPAIR - 1)
                              for p in range(NPAIR - 1):
                                  for plane in range(2):
                                      nc.tensor.matmul(
                                          psx[:], dall[:, p],
                                          pair_rhs(sf_cur[mc], bl, plane,
                                                   p, r0, r1),
                                          start=(nmm == 0), stop=False,
                                          perf_mode=DR,
                                      )
                                      nmm += 1
                              nc.tensor.matmul(
                                  psx[:], dall[:, NPAIR - 1],
                                  pair_rhs(sf_cur[mc], bl, 0, NPAIR - 1,
                                           r0, r1),
                                  start=False, stop=True, perf_mode=DR,
                              )
                              csl = slice(r0 * HO, r1 * HO)
                              nc.scalar.activation(ftall[:, mc, 0, csl],
                                                   psx[:], copyfn)
                              nc.vector.tensor_tensor(
                                  ftall[:, mc, 1, csl], psx[:],
                                  ftall[:, mc, 0, csl],
                                  mybir.AluOpType.subtract)

                          if mc < NCH - 1:
                              continue

                          # head conv1 (fp8 three-term DR) + BN/ReLU
                          for mq in range(NCH):
                              msl = slice(mq * 128, (mq + 1) * 128)
                              yt = apool.tile([128, HO * HO], bf16,
                                              name=f"y{mq}", tag=f"y{mq}",
                                              bufs=2)
                              for ph, (c0, c1) in enumerate(((0, 500),
                                                            (500, 625))):
                                  ps = ppool.tile([128, c1 - c0], f32,
                                                  name="ps_hd",
                                                  tag=("hdA" if ph == 0
                                                       else "hdB"),
                                                  bufs=(2 if ph == 0 else 1))
                                  terms = (
                                      (wh1_t["wh1_hi"], 0),
                                      (wh1_t["wh1_hi"], 1),
                                      (wh1_t["wh1_lo"], 0),
                                  )
                                  for j, (wt, plane) in enumerate(terms):
                                      nc.tensor.matmul(
                                          ps[:], wt[:, :, msl],
                                          ftall[:, :, plane, c0:c1],
                                          start=(j == 0), stop=(j == 2),
                                          perf_mode=DR,
                                      )
                                  nc.scalar.activation(yt[:, c0:c1], ps[:],
                                                       relu, bias=shh_t[mq][:],
                                                       scale=sch_t[mq][:])
                              ys[mq] = yt

                          # head conv2 + bias (bf16)
                          ot = apool.tile([COUT, HO * HO], f32,
                                          name="ot", tag="ot", bufs=2)
                          for ph, (c0, c1) in enumerate(((0, 500), (500, 625))):
                              ps = ppool.tile([COUT, c1 - c0], f32,
                                              name="ps_o",
                                              tag=("hdA" if ph == 0 else "hdB"),
                                              bufs=(2 if ph == 0 else 1))
                              for kc in range(NCH):
                                  nc.tensor.matmul(
                                      ps[:],
                                      wh2_t[kc][:],
                                      ys[kc][:, c0:c1],
                                      start=(kc == 0), stop=(kc == NCH - 1),
                                  )
                              nc.scalar.activation(ot[:, c0:c1], ps[:], idfn,
                                                   bias=bh2_t[:], scale=1.0)
                              nc.sync.dma_start(
                                  out[b_abs][:].rearrange(
                                      "o h w -> o (h w)")[:, c0:c1],
                                  ot[:, c0:c1])
                  if sf_next is not None:
                      sf_cur = sf_next

    nc.compile()
    return nc


def _get_nc():
    if "nc" not in _CACHE:
        _CACHE["nc"] = _build_nc()
    return _CACHE["nc"]


def _split8(a):
    hi = a.astype(E4M3)
    lo = (a - hi.astype(np.float32)).astype(E4M3)
    return hi, lo


def _prep_in_maps(inputs):
    """Host-side input prep shared by kernel() and the test harness."""
    def fold(g, b, m, v):
        sc = (np.asarray(g) / np.sqrt(np.asarray(v) + EPS)).astype(np.float32)
        sh = (np.asarray(b) - np.asarray(m) * sc).astype(np.float32)
        return sc.reshape(-1, 1), sh.reshape(-1, 1)

    sck, shk = fold(inputs["g_k"], inputs["b_k"], inputs["m_k"], inputs["v_k"])
    scs, shs = fold(inputs["g_s"], inputs["b_s"], inputs["m_s"], inputs["v_s"])
    sch, shh = fold(inputs["g_h"], inputs["b_h"], inputs["m_h"], inputs["v_h"])

    w_k = np.asarray(inputs["w_k"])
    w_s = np.asarray(inputs["w_s"])
    w_h1 = np.asarray(inputs["w_h1"])
    w_h2 = np.asarray(inputs["w_h2"])
    bias_h2 = np.asarray(inputs["bias_h2"])

    ws16 = (np.ascontiguousarray(w_s.T).astype(BF16).astype(np.float32)
            * WSCALE)
    ws_hi, ws_lo = _split8(ws16)
    wh116 = (np.ascontiguousarray(w_h1.T).astype(BF16).astype(np.float32)
             * WSCALE)
    wh1_hi, wh1_lo = _split8(wh116)

    common = {
        "wkT": np.ascontiguousarray(w_k.T).astype(BF16),
        "ws_hi": ws_hi, "ws_lo": ws_lo,
        "wh1_hi": wh1_hi, "wh1_lo": wh1_lo,
        "wh2T": np.ascontiguousarray(w_h2.T).astype(BF16),
        "sck": sck, "shk": shk,
        "scs": scs / WSCALE, "shs": shs,
        "sch": sch / WSCALE, "shh": shh,
        "bh2": bias_h2.astype(np.float32).reshape(-1, 1),
        "ident": np.eye(128, dtype=BF16),
    }
    xk_all = np.asarray(inputs["kernel"]).astype(BF16)
    xs16 = np.asarray(inputs["search"]).astype(BF16).astype(np.float32)
    xs_hi_all, xs_lo_all = _split8(xs16)

    in_maps = []
    for i in range(NCORES):
        bs = slice(i * NB, (i + 1) * NB)
        m = dict(common)
        m["xk"] = np.ascontiguousarray(xk_all[bs])
        m["xs_hi"] = np.ascontiguousarray(xs_hi_all[bs])
        m["xs_lo"] = np.ascontiguousarray(xs_lo_all[bs])
        in_maps.append(m)
    return in_maps


def kernel(kernel, search, w_k, g_k, b_k, m_k, v_k, w_s, g_s, b_s, m_s, v_s,
           w_h1, g_h, b_h, m_h, v_h, w_h2, bias_h2):
    from concourse.bass_utils import run_bass_kernel_spmd

    inputs = dict(kernel=kernel, search=search, w_k=w_k, g_k=g_k, b_k=b_k,
                  m_k=m_k, v_k=v_k, w_s=w_s, g_s=g_s, b_s=b_s, m_s=m_s,
                  v_s=v_s, w_h1=w_h1, g_h=g_h, b_h=b_h, m_h=m_h, v_h=v_h,
                  w_h2=w_h2, bias_h2=bias_h2)
    in_maps = _prep_in_maps(inputs)
    nc = _get_nc()
    last_err = None
    for _attempt in range(2):
        try:
            res = run_bass_kernel_spmd(nc, in_maps,
                                       core_ids=list(range(NCORES)))
            return np.concatenate(
                [res.results[i]["out"] for i in range(NCORES)], axis=0)
        except Exception as e:  # transient axon/mesh errors: retry
            last_err = e
    # Fallback: the batch shards are independent (no collectives), so a
    # wedged multi-core mesh can be routed around with per-core dispatches.
    try:
        outs = []
        for i in range(NCORES):
            res = run_bass_kernel_spmd(nc, [in_maps[i]], core_ids=[i])
            outs.append(res.results[0]["out"])
        return np.concatenate(outs, axis=0)
    except Exception:
        raise last_err
